# revision 1
# baseline (speedup 1.0000x reference)
"""Trainium2 Bass kernel for nn_Decoder (2-layer LSTM decoder with BatchNorm +
LockedDropout + vocab projection), tensor-parallel over the hidden dim across
8 NeuronCores.

Contract: kernel(**inputs) takes FULL inputs (as produced by setup_inputs())
and returns the FULL [B*T, V] float32 output.

Sharding:
  - Each core owns a 128-unit slice of the hidden dim for BOTH LSTM layers
    (gates i,f,g,o for those units) -> gate matmuls have M=128 per gate with
    full batch B=256 as the moving dim (full PE width, BN stats exact).
  - Recurrent state h1/h2 is all-gathered across cores every step (ncfw
    AllGather through HBM).  y1 rides with h1; y2 rides with h2.
  - The vocab projection is sharded over V (1250 per core) and interleaved
    into the recurrence loop (step t projects step t-1's y2) so TensorE
    stays busy during collective waits.
  - Matmuls run in float32r (full PE rate, ~13-bit mantissa).  Weights and
    x are pre-rounded to fp32r values on the host; on-chip producers write
    float32r directly.
"""

import contextlib
import os
import sys

sys.path.insert(0, "/opt/trn_rl_repo")

import ml_dtypes
import numpy as np

import concourse.bass as bass
import concourse.tile as tile
from concourse import bacc, mybir
from concourse.bass_utils import run_bass_kernel_spmd

F32 = mybir.dt.float32
F32R = mybir.dt.float32r
BF16 = mybir.dt.bfloat16

# matmul/transport dtype: "f32r" (full-rate, near-fp32), "bf16", "f32" (4x slower)
DT_MM_NAME = os.environ.get("TRN_DT_MM", "f32r")
DT_MM = {"f32r": F32R, "bf16": BF16, "f32": F32}[DT_MM_NAME]

B, L, E, H, V = 256, 20, 512, 1024, 10000
T = int(os.environ.get("TRN_T", L + 1))
NCORE = 8
P = 128
HS = H // NCORE          # 128 hidden units per core per layer
VS = V // NCORE          # 1250 vocab slots per core
NKE = E // P             # 4 k-tiles over E
NKH = H // P             # 8 k-tiles over H
BN_EPS = 1e-5
# projection N-chunks of VS=1250 (each >=256 so fp32r runs full rate;
# fp32r requires even N and 8-byte-aligned dst start)
NCHUNKS = [(0, 418), (418, 416), (834, 416)]

LAST_EXEC_NS = None
# TRN_FAKE_AG=1 replaces collectives with a local DMA (timing-model runs only)
FAKE_AG = os.environ.get("TRN_FAKE_AG", "0") == "1"

_CACHE = {}


def _fp32r_round(x):
    """Round fp32 -> nearest fp32r value (sum of two bf16s), like the
    hardware's rounding producers / walrus cast_fp32_to_fp32r."""
    hi = x.astype(ml_dtypes.bfloat16).astype(np.float32)
    lo = (x - hi).astype(ml_dtypes.bfloat16).astype(np.float32)
    return hi + lo


def build_bass():
    nc = bacc.Bacc("TRN2", target_bir_lowering=False, num_devices=NCORE)
    dt = DT_MM
    cast = dt == BF16           # on-chip convert path (bf16 only)
    wdt = F32 if dt == F32 else (F32R if dt == F32R else F32)
    # DRAM dtype for weights/x: fp32r values are host-pre-rounded, so the
    # tensors are declared fp32r and DMA'd straight into fp32r tiles.
    ddt = F32R if dt == F32R else F32

    # ---------------- DRAM I/O ----------------
    d_xT = nc.dram_tensor("xT", [T, NKE, P, B], ddt, kind="ExternalInput")
    d_wih1 = nc.dram_tensor("wih1", [4, NKE, P, HS], ddt, kind="ExternalInput")
    d_whh1 = nc.dram_tensor("whh1", [4, NKH, P, HS], ddt, kind="ExternalInput")
    d_wih2 = nc.dram_tensor("wih2", [4, NKH, P, HS], ddt, kind="ExternalInput")
    d_whh2 = nc.dram_tensor("whh2", [4, NKH, P, HS], ddt, kind="ExternalInput")
    d_woutT = nc.dram_tensor("woutT", [NKH, P, VS], ddt, kind="ExternalInput")
    d_bias1 = nc.dram_tensor("bias1", [HS, 4], F32, kind="ExternalInput")
    d_bias2 = nc.dram_tensor("bias2", [HS, 4], F32, kind="ExternalInput")
    d_gb1 = nc.dram_tensor("gb1", [HS, 2], F32, kind="ExternalInput")
    d_gb2 = nc.dram_tensor("gb2", [HS, 2], F32, kind="ExternalInput")
    d_m1T = nc.dram_tensor("m1T", [HS, B], F32, kind="ExternalInput")
    d_m2T = nc.dram_tensor("m2T", [HS, B], F32, kind="ExternalInput")
    d_out = nc.dram_tensor("out", [B * T, VS], F32, kind="ExternalOutput")
    # out rows are (b, t) packed; view for per-(t, b-block) strided writes
    d_out_r = d_out[:].rearrange("(b t) v -> b t v", t=T)

    # collective bounce buffers (inputs must be Local, outputs Shared)
    RING = 3
    ag1i = [nc.dram_tensor(f"ag1i{j}", [2 * P, B], dt, kind="Internal")
            for j in range(RING)]
    ag1o = [nc.dram_tensor(f"ag1o{j}", [2 * P * NCORE, B], dt,
                           kind="Internal", addr_space="Shared")
            for j in range(RING)]
    ag2i = [nc.dram_tensor(f"ag2i{j}", [2 * P, B], dt, kind="Internal")
            for j in range(RING)]
    ag2o = [nc.dram_tensor(f"ag2o{j}", [2 * P * NCORE, B], dt,
                           kind="Internal", addr_space="Shared")
            for j in range(RING)]

    dma = nc.sync.dma_start

    with tile.TileContext(nc) as tc:
        with contextlib.ExitStack() as ctx:
            smalls = ctx.enter_context(tc.tile_pool(name="smalls", bufs=1))
            wts = ctx.enter_context(tc.tile_pool(name="wts", bufs=1))
            stage = ctx.enter_context(tc.tile_pool(name="stage", bufs=2))
            xpool = ctx.enter_context(tc.tile_pool(name="xpool", bufs=2))
            gp_h1 = ctx.enter_context(tc.tile_pool(name="g_h1", bufs=1))
            gp_y1 = ctx.enter_context(tc.tile_pool(name="g_y1", bufs=1))
            gp_y2 = ctx.enter_context(tc.tile_pool(name="g_y2", bufs=2))
            cell = ctx.enter_context(tc.tile_pool(name="cell", bufs=3))
            slpool = ctx.enter_context(tc.tile_pool(name="slp", bufs=2))
            state = ctx.enter_context(tc.tile_pool(name="state", bufs=1))
            psumg = ctx.enter_context(
                tc.tile_pool(name="psumg", bufs=5, space="PSUM"))
            psumP = ctx.enter_context(
                tc.tile_pool(name="psumP", bufs=3, space="PSUM"))
            outp = ctx.enter_context(tc.tile_pool(name="outp", bufs=6))

            # small constants
            b1 = smalls.tile([HS, 4], F32)
            b2 = smalls.tile([HS, 4], F32)
            gb1 = smalls.tile([HS, 2], F32)
            gb2 = smalls.tile([HS, 2], F32)
            m1 = smalls.tile([HS, B], F32)
            m2 = smalls.tile([HS, B], F32)
            epst = smalls.tile([P, 1], F32)
            for dst, src in ((b1, d_bias1), (b2, d_bias2), (gb1, d_gb1),
                             (gb2, d_gb2), (m1, d_m1T), (m2, d_m2T)):
                dma(dst[:], src[:])
            nc.vector.memset(epst[:], BN_EPS)

            # resident weights
            w_ih1 = wts.tile([P, 4, NKE, HS], dt)
            w_hh1 = wts.tile([P, 4, NKH, HS], dt)
            w_ih2 = wts.tile([P, 4, NKH, HS], dt)
            w_hh2 = wts.tile([P, 4, NKH, HS], dt)
            w_out = wts.tile([P, NKH, VS], dt)

            def load_weight(dst, dram, n_g, n_k):
                # dst [P, n_g, n_k, HS]; dram [n_g, n_k, P, HS]
                for g in range(n_g):
                    if cast:
                        st = stage.tile([P, n_k, HS], F32, tag="wstage",
                                        name=f"wst_{dram.name}_{g}")
                        dma(st[:], dram[g][:].rearrange("k p m -> p k m"))
                        nc.vector.tensor_copy(dst[:, g], st[:])
                    else:
                        dma(dst[:, g], dram[g][:].rearrange("k p m -> p k m"))

            load_weight(w_ih1, d_wih1, 4, NKE)
            load_weight(w_hh1, d_whh1, 4, NKH)
            load_weight(w_ih2, d_wih2, 4, NKH)
            load_weight(w_hh2, d_whh2, 4, NKH)
            for k in range(NKH):
                if cast:
                    st = stage.tile([P, VS], F32, tag="pstage",
                                    name=f"wst_wout_{k}")
                    dma(st[:], d_woutT[k][:])
                    nc.vector.tensor_copy(w_out[:, k], st[:])
                else:
                    dma(w_out[:, k], d_woutT[k][:])

            # persistent state
            c1 = state.tile([P, B], F32)
            c2 = state.tile([P, B], F32)
            nc.vector.memset(c1[:], 0.0)
            nc.vector.memset(c2[:], 0.0)

            def lstm_cell(emit_gates, bias, gbv, mask, c_st,
                          h_out, y_out, t, pgs=None):
                """One LSTM cell + BatchNorm + dropout-mask.

                psum packing: pgA=(i,g), pgB=(f,o); gate order i=0 f=1 g=2 o=3.
                pgs: pre-allocated (pgA, pgB) whose accumulation was already
                started (x-side matmuls emitted in the previous step).
                """
                if pgs is None:
                    pgA = psumg.tile([P, 2, B], F32, tag="pg", name=f"pgA_{t}")
                    pgB = psumg.tile([P, 2, B], F32, tag="pg", name=f"pgB_{t}")
                else:
                    pgA, pgB = pgs
                gloc = {0: (pgA, 0), 2: (pgA, 1), 1: (pgB, 0), 3: (pgB, 1)}
                for gate in (0, 2, 1, 3):
                    tl, sub = gloc[gate]
                    emit_gates(gate, tl[:, sub])

                i_t = cell.tile([P, B], F32, tag="i", name=f"i_{t}")
                f_t = cell.tile([P, B], F32, tag="f", name=f"f_{t}")
                g_t = cell.tile([P, B], F32, tag="g", name=f"g_{t}")
                o_t = cell.tile([P, B], F32, tag="o", name=f"o_{t}")
                Sig = mybir.ActivationFunctionType.Sigmoid
                Tanh = mybir.ActivationFunctionType.Tanh
                nc.scalar.activation(i_t[:], pgA[:, 0], Sig, bias=bias[:, 0:1])
                nc.scalar.activation(g_t[:], pgA[:, 1], Tanh, bias=bias[:, 2:3])
                nc.scalar.activation(f_t[:], pgB[:, 0], Sig, bias=bias[:, 1:2])
                nc.scalar.activation(o_t[:], pgB[:, 1], Sig, bias=bias[:, 3:4])

                ig = cell.tile([P, B], F32, tag="ig", name=f"ig_{t}")
                nc.vector.tensor_mul(ig[:], i_t[:], g_t[:])
                fc = cell.tile([P, B], F32, tag="fc", name=f"fc_{t}")
                nc.vector.tensor_mul(fc[:], f_t[:], c_st[:])
                nc.vector.tensor_add(c_st[:], ig[:], fc[:])
                tnc = cell.tile([P, B], F32, tag="tc", name=f"tc_{t}")
                nc.scalar.activation(tnc[:], c_st[:], Tanh)
                h_f = cell.tile([P, B], F32, tag="h", name=f"h_{t}")
                nc.vector.tensor_mul(h_f[:], o_t[:], tnc[:])
                if dt != F32:
                    nc.vector.tensor_copy(h_out[:], h_f[:])
                # BN stats over batch (free dim)
                st6 = cell.tile([P, 6], F32, tag="st", name=f"st_{t}")
                nc.vector.bn_stats(st6[:], h_f[:])
                mv = cell.tile([P, 2], F32, tag="mv", name=f"mv_{t}")
                nc.vector.bn_aggr(mv[:], st6[:])
                # rstd = rsqrt(var + eps), DVE-only (fast-inverse-sqrt +
                # 3 Newton steps) -- keeps the ACT LUT on sigmoid/tanh, no
                # table swaps.
                I32 = mybir.dt.int32
                v_t = cell.tile([P, 1], F32, tag="vv", name=f"vv_{t}")
                nc.vector.tensor_scalar_add(v_t[:], mv[:, 1:2], BN_EPS)
                r_a = cell.tile([P, 1], F32, tag="ra", name=f"ra_{t}")
                r_b = cell.tile([P, 1], F32, tag="rb", name=f"rb_{t}")
                ui = cell.tile([P, 1], I32, tag="ui", name=f"ui_{t}")
                nc.vector.tensor_scalar(ui[:], v_t[:].bitcast(I32), 1, None,
                                        op0=mybir.AluOpType.logical_shift_right)
                nc.vector.tensor_scalar(r_a[:].bitcast(I32), ui[:],
                                        -1, 0x5F3759DF,
                                        op0=mybir.AluOpType.mult,
                                        op1=mybir.AluOpType.add)
                rr = cell.tile([P, 1], F32, tag="rr", name=f"rr_{t}")
                ww = cell.tile([P, 1], F32, tag="ww", name=f"ww_{t}")
                r_cur, r_nxt = r_a, r_b
                for it in range(2):
                    nc.vector.tensor_mul(rr[:], r_cur[:], r_cur[:])
                    nc.vector.scalar_tensor_tensor(
                        ww[:], rr[:], -0.5, v_t[:],
                        op0=mybir.AluOpType.mult, op1=mybir.AluOpType.mult)
                    nc.vector.scalar_tensor_tensor(
                        r_nxt[:], ww[:], 1.5, r_cur[:],
                        op0=mybir.AluOpType.add, op1=mybir.AluOpType.mult)
                    r_cur, r_nxt = r_nxt, r_cur
                a_v = cell.tile([P, 1], F32, tag="av", name=f"av_{t}")
                nc.vector.tensor_mul(a_v[:], r_cur[:], gbv[:, 0:1])
                ma = cell.tile([P, 1], F32, tag="ma", name=f"ma_{t}")
                nc.vector.tensor_mul(ma[:], mv[:, 0:1], a_v[:])
                b_v = cell.tile([P, 1], F32, tag="bv", name=f"bv_{t}")
                nc.vector.tensor_sub(b_v[:], gbv[:, 1:2], ma[:])
                yt = cell.tile([P, B], F32, tag="yt", name=f"yt_{t}")
                nc.scalar.activation(yt[:], h_f[:],
                                     mybir.ActivationFunctionType.Identity,
                                     bias=b_v[:], scale=a_v[:])
                nc.vector.tensor_mul(y_out[:], yt[:], mask[:])
                return h_f

            def project(tp, y2g_src):
                """Projection of step tp's y2 (gathered in y2g_src)."""
                for bh in range(2):
                    pps = []
                    for n in range(len(NCHUNKS)):
                        pps.append(psumP.tile([P, 512], F32, tag="pp",
                                              name=f"pp_{tp}_{bh}_{n}"))
                    for k in range(NKH):
                        lhs = y2g_src[:, k, bh * P:(bh + 1) * P]
                        for n, (noff, nlen) in enumerate(NCHUNKS):
                            nc.tensor.matmul(
                                pps[n][:, 0:nlen], lhs,
                                w_out[:, k, noff:noff + nlen],
                                start=(k == 0), stop=(k == NKH - 1))
                    for n, (noff, nlen) in enumerate(NCHUNKS):
                        o_sb = outp.tile([P, 512], F32, tag="osb",
                                         name=f"osb_{tp}_{bh}_{n}")
                        nc.vector.tensor_copy(o_sb[:, 0:nlen],
                                              pps[n][:, 0:nlen])
                        dma(d_out_r[bh * P:(bh + 1) * P, tp,
                                    noff:noff + nlen], o_sb[:, 0:nlen])

            h1g = None
            h2g = None
            y2g_prev = None

            def load_x(t):
                x_t = xpool.tile([P, NKE, B], dt, tag="x", name=f"x_{t}")
                if cast:
                    xs = xpool.tile([P, NKE, B], F32, tag="xs", name=f"xs_{t}")
                    dma(xs[:], d_xT[t][:].rearrange("k p b -> p k b"))
                    nc.vector.tensor_copy(x_t[:], xs[:])
                else:
                    dma(x_t[:], d_xT[t][:].rearrange("k p b -> p k b"))
                return x_t

            def emit_ih1(t, x_t, stop):
                # x-side of layer-1 gates for step t.  One accumulation group
                # per PSUM bank: start on the bank's first matmul (gates 0/1),
                # stop on its last (gates 2/3; deferred to the hh matmuls
                # unless `stop`).  Gate g's first matmul uses start=False --
                # its half-bank has_written bits are clear, so it overwrites.
                pgA = psumg.tile([P, 2, B], F32, tag="pg", name=f"pgA_1_{t}")
                pgB = psumg.tile([P, 2, B], F32, tag="pg", name=f"pgB_1_{t}")
                gloc = {0: (pgA, 0), 2: (pgA, 1), 1: (pgB, 0), 3: (pgB, 1)}
                for gate in (0, 2, 1, 3):
                    tl, sub = gloc[gate]
                    for k in range(NKE):
                        nc.tensor.matmul(
                            tl[:, sub], w_ih1[:, gate, k], x_t[:, k],
                            start=(k == 0 and gate in (0, 1)),
                            stop=(stop and k == NKE - 1 and gate in (2, 3)))
                return pgA, pgB

            x_t = load_x(0)
            pending_l1 = emit_ih1(0, x_t, stop=True)

            for t in range(T):
                # ---- layer 1 (x-side already emitted; add hh1) ----
                def l1_gates(gate, pap, h1g=h1g, t=t):
                    if t > 0:
                        for k in range(NKH):
                            nc.tensor.matmul(
                                pap, w_hh1[:, gate, k], h1g[:, k], start=False,
                                stop=(k == NKH - 1 and gate in (2, 3)))

                h1_sl = slpool.tile([P, B], dt, tag="h1s", name=f"h1s_{t}")
                y1_sl = slpool.tile([P, B], dt, tag="y1s", name=f"y1s_{t}")
                h1f = lstm_cell(l1_gates, b1, gb1, m1, c1,
                                h1_sl, y1_sl, f"1_{t}", pgs=pending_l1)
                if dt == F32:
                    h1_sl = h1f

                # ---- AllGather (h1 ; y1) ----
                agi = ag1i[t % RING]
                ago = ag1o[t % RING]
                dma(agi[0:P], h1_sl[:])
                dma(agi[P:2 * P], y1_sl[:])
                if FAKE_AG:
                    dma(ago[0:2 * P], agi[:])
                else:
                    nc.gpsimd.collective_compute(
                        "AllGather", mybir.AluOpType.bypass,
                        ins=[agi[:].opt()], outs=[ago[:].opt()],
                        replica_groups=[list(range(NCORE))])
                agor = ago[:].rearrange("(c two p) b -> two p c b", two=2, p=P)
                h1g = gp_h1.tile([P, NCORE, B], dt, tag="h1g", name=f"h1g_{t}")
                dma(h1g[:], agor[0])
                y1g = gp_y1.tile([P, NCORE, B], dt, tag="y1g", name=f"y1g_{t}")
                for k in range(NCORE):
                    dma(y1g[:, k], agor[1, :, k])

                # ---- projection of step t-1 (fills the AG wait) ----
                if y2g_prev is not None:
                    project(t - 1, y2g_prev)

                # ---- pre-emit next step's x-side gate matmuls (more PE
                # fill work during this step's gather waits) ----
                if t + 1 < T:
                    x_nxt = load_x(t + 1)
                    pending_l1 = emit_ih1(t + 1, x_nxt, stop=False)

                # ---- layer 2 ----
                # hh2 first (h2g already gathered last step) so layer 2
                # starts before y1g lands; ih2 accumulates after.
                def l2_gates(gate, pap, y1g=y1g, h2g=h2g, t=t):
                    if t > 0:
                        for k in range(NKH):
                            nc.tensor.matmul(pap, w_hh2[:, gate, k], h2g[:, k],
                                             start=(k == 0), stop=False)
                    for k in range(NKH):
                        nc.tensor.matmul(pap, w_ih2[:, gate, k], y1g[:, k],
                                         start=(t == 0 and k == 0),
                                         stop=(k == NKH - 1))

                h2_sl = slpool.tile([P, B], dt, tag="h2s", name=f"h2s_{t}")
                y2_sl = slpool.tile([P, B], dt, tag="y2s", name=f"y2s_{t}")
                h2f = lstm_cell(l2_gates, b2, gb2, m2, c2,
                                h2_sl, y2_sl, f"2_{t}")
                if dt == F32:
                    h2_sl = h2f

                # ---- AllGather (h2 ; y2) ----
                agi = ag2i[t % RING]
                ago = ag2o[t % RING]
                dma(agi[0:P], h2_sl[:])
                dma(agi[P:2 * P], y2_sl[:])
                if FAKE_AG:
                    dma(ago[0:2 * P], agi[:])
                else:
                    nc.gpsimd.collective_compute(
                        "AllGather", mybir.AluOpType.bypass,
                        ins=[agi[:].opt()], outs=[ago[:].opt()],
                        replica_groups=[list(range(NCORE))])
                agor2 = ago[:].rearrange("(c two p) b -> two p c b",
                                         two=2, p=P)
                hy2g = gp_y2.tile([P, 2, NCORE, B], dt, tag="hy2g",
                                  name=f"hy2g_{t}")
                if t < T - 1:
                    dma(hy2g[:, 0], agor2[0])
                dma(hy2g[:, 1], agor2[1])
                h2g = hy2g[:, 0]
                y2g_prev = hy2g[:, 1]

            # tail projection for the last step
            project(T - 1, y2g_prev)

    nc.finalize()
    return nc


def _prep_inputs(features, captions, lengths, embed_table,
                 W_ih1, W_hh1, b_ih1, b_hh1, gamma1, beta1, mask1,
                 W_ih2, W_hh2, b_ih2, b_hh2, gamma2, beta2, mask2,
                 W_out, b_out):
    f32 = np.float32
    features = np.asarray(features, f32)
    captions = np.asarray(captions)
    embed_table = np.asarray(embed_table, f32)
    rnd = _fp32r_round if DT_MM == F32R else (lambda a: a)

    # x sequence [T, B, E] -> xT [T, NKE, P, B]
    x = np.empty((L + 1, B, E), f32)
    x[0] = features
    x[1:] = embed_table[captions].transpose(1, 0, 2)
    x = x[:T]
    xT = rnd(np.ascontiguousarray(x.transpose(0, 2, 1).reshape(T, NKE, P, B)))

    def wslice(Wf, c, K):
        # Wf [4H, K] -> per-core [4, K//P, P, HS] lhsT blocks
        Wg = np.asarray(Wf, f32).reshape(4, H, K)[:, c * HS:(c + 1) * HS, :]
        # out[g, k, kk, m] = Wg[g, m, k*P + kk]
        return rnd(np.ascontiguousarray(
            Wg.transpose(0, 2, 1).reshape(4, K // P, P, HS)))

    bsum1 = (np.asarray(b_ih1, f32) + np.asarray(b_hh1, f32)).reshape(4, H)
    bsum2 = (np.asarray(b_ih2, f32) + np.asarray(b_hh2, f32)).reshape(4, H)
    WoT = np.ascontiguousarray(np.asarray(W_out, f32).T)  # [H, V]

    in_maps = []
    for c in range(NCORE):
        u = slice(c * HS, (c + 1) * HS)
        v = slice(c * VS, (c + 1) * VS)
        in_maps.append({
            "xT": xT,
            "wih1": wslice(W_ih1, c, E),
            "whh1": wslice(W_hh1, c, H),
            "wih2": wslice(W_ih2, c, H),
            "whh2": wslice(W_hh2, c, H),
            "woutT": rnd(np.ascontiguousarray(
                WoT[:, v].reshape(NKH, P, VS))),
            "bias1": np.ascontiguousarray(bsum1[:, u].T),
            "bias2": np.ascontiguousarray(bsum2[:, u].T),
            "gb1": np.ascontiguousarray(
                np.stack([np.asarray(gamma1, f32)[u],
                          np.asarray(beta1, f32)[u]], axis=1)),
            "gb2": np.ascontiguousarray(
                np.stack([np.asarray(gamma2, f32)[u],
                          np.asarray(beta2, f32)[u]], axis=1)),
            "m1T": np.ascontiguousarray(np.asarray(mask1, f32).T[u]),
            "m2T": np.ascontiguousarray(np.asarray(mask2, f32).T[u]),
        })
    return in_maps, np.asarray(b_out, f32)


def kernel(**inputs):
    global LAST_EXEC_NS
    if "nc" not in _CACHE:
        _CACHE["nc"] = build_bass()
    nc = _CACHE["nc"]

    in_maps, b_out = _prep_inputs(**inputs)
    trace = os.environ.get("TRN_KERNEL_TRACE", "0") == "1"
    res = run_bass_kernel_spmd(nc, in_maps, core_ids=list(range(NCORE)),
                               trace=trace)
    LAST_EXEC_NS = res.exec_time_ns
    out = np.concatenate([res.results[c]["out"] for c in range(NCORE)], axis=1)
    if b_out.any():
        out = out + b_out[None, :]
    return out



# revision 17
# speedup vs baseline: 1.6974x; 1.6974x over previous
"""Trainium2 Bass kernel for nn_Decoder (2-layer LSTM decoder with BatchNorm +
LockedDropout + vocab projection), tensor-parallel over the hidden dim across
8 NeuronCores.

Contract: kernel(**inputs) takes FULL inputs (as produced by setup_inputs())
and returns the FULL [B*T, V] float32 output.

Sharding / schedule (v3):
  - Each core owns a 128-unit slice of the hidden dim for BOTH LSTM layers
    (full batch B=256 moving dim, exact BN stats) and a 1250-wide slice of
    the vocab projection.
  - ONE AllGather per step (22 total): AG_j carries (h1_j, y1_j, h2_{j-1},
    y2_{j-1}).  Layer 2 of step j runs after AG_j lands, together with
    layer 1 of step j+1 -> a single collective round trip per step.
  - The vocab projection of step j-1 is emitted right after AG_j launches,
    so the PE has ~10us of AG-independent work during every collective
    wait (keeps the PE pstate ramped, hides AG latency).
  - Transport dtypes: h1/y1/h2 ride the collective in bf16 (halves the
    gather traffic; they feed gate matmuls as the *moving* operand).  y2
    rides in f32r, byte-packed into the bf16 buffer, because it is the
    *stationary* operand of the projection (f32r stationary emits no
    Ldweights instruction and keeps full precision on the output path).
  - Gather-in DMAs are per-slot, priority-ordered (h2, y1, h1, y2), with
    half-k splits so layer-2 gates start after half a slot lands.
  - PSUM->SBUF projection copies run on the ACT engine; projection output
    DMAs are issued from the Pool queue (SWDGE) to keep the shared HWDGE
    path free for x-loads and gathers.
"""

import contextlib
import os
import sys

sys.path.insert(0, "/opt/trn_rl_repo")

import ml_dtypes
import numpy as np

import concourse.bass as bass
import concourse.tile as tile
from concourse import bacc, mybir
from concourse.bass_utils import run_bass_kernel_spmd

F32 = mybir.dt.float32
F32R = mybir.dt.float32r
BF16 = mybir.dt.bfloat16

B, L, E, H, V = 256, 20, 512, 1024, 10000
T = int(os.environ.get("TRN_T", L + 1))
NCORE = 8
P = 128
HS = H // NCORE          # 128 hidden units per core per layer
VS = V // NCORE          # 1250 vocab slots per core
NKE = E // P             # 4 k-tiles over E
NKH = H // P             # 8 k-tiles over H
BN_EPS = 1e-5
RING = 3
NR_ITERS = int(os.environ.get("TRN_NR", "1"))
# projection N-chunks of VS=1250 (even lengths; fp32r full rate needs >=256)
NCHUNKS = [(0, 418), (418, 416), (834, 416)]
# agi rows (bf16 [6P, B]): h1 [0:P], y1 [P:2P], h2 [2P:3P], pad [3P:4P],
# y2 f32-byte-image [4P:6P]
AGROWS = 6 * P

LAST_EXEC_NS = None
# TRN_FAKE_AG=1 replaces collectives with a local DMA (timing-model runs only)
FAKE_AG = os.environ.get("TRN_FAKE_AG", "0") == "1"

_CACHE = {}


def _fp32r_round(x):
    """Round fp32 -> nearest fp32r value (sum of two bf16s)."""
    hi = x.astype(ml_dtypes.bfloat16).astype(np.float32)
    lo = (x - hi).astype(ml_dtypes.bfloat16).astype(np.float32)
    return hi + lo


def _to_bf16(x):
    return x.astype(ml_dtypes.bfloat16)


def build_bass():
    nc = bacc.Bacc("TRN2", target_bir_lowering=False, num_devices=NCORE)

    # ---------------- DRAM I/O ----------------
    d_xT = nc.dram_tensor("xT", [T, NKE, P, B], F32R, kind="ExternalInput")
    d_wih1 = nc.dram_tensor("wih1", [4, NKE, P, HS], F32R, kind="ExternalInput")
    d_whh1 = nc.dram_tensor("whh1", [4, NKH, P, HS], BF16, kind="ExternalInput")
    d_wih2 = nc.dram_tensor("wih2", [4, NKH, P, HS], BF16, kind="ExternalInput")
    d_whh2 = nc.dram_tensor("whh2", [4, NKH, P, HS], BF16, kind="ExternalInput")
    d_woutT = nc.dram_tensor("woutT", [NKH, P, VS], F32R, kind="ExternalInput")
    d_bias1 = nc.dram_tensor("bias1", [HS, 4], F32, kind="ExternalInput")
    d_bias2 = nc.dram_tensor("bias2", [HS, 4], F32, kind="ExternalInput")
    d_gb1 = nc.dram_tensor("gb1", [HS, 2], F32, kind="ExternalInput")
    d_gb2 = nc.dram_tensor("gb2", [HS, 2], F32, kind="ExternalInput")
    d_m1T = nc.dram_tensor("m1T", [HS, B], F32, kind="ExternalInput")
    d_m2T = nc.dram_tensor("m2T", [HS, B], F32, kind="ExternalInput")
    d_out = nc.dram_tensor("out", [B * T, VS], F32, kind="ExternalOutput")
    # out rows are (b, t) packed; view for per-(t, b-block) strided writes
    d_out_r = d_out[:].rearrange("(b t) v -> b t v", t=T)

    agi = [nc.dram_tensor(f"agi{r}", [AGROWS, B], BF16, kind="Internal")
           for r in range(RING)]
    ago = [nc.dram_tensor(f"ago{r}", [AGROWS * NCORE, B], BF16,
                          kind="Internal", addr_space="Shared")
           for r in range(RING)]

    dma = nc.sync.dma_start
    Sig = mybir.ActivationFunctionType.Sigmoid
    Tanh = mybir.ActivationFunctionType.Tanh
    Ident = mybir.ActivationFunctionType.Identity
    I32 = mybir.dt.int32

    with tile.TileContext(nc) as tc:
        with contextlib.ExitStack() as ctx:
            smalls = ctx.enter_context(tc.tile_pool(name="smalls", bufs=1))
            wts = ctx.enter_context(tc.tile_pool(name="wts", bufs=1))
            xpool = ctx.enter_context(tc.tile_pool(name="xpool", bufs=2))
            gpool = ctx.enter_context(tc.tile_pool(name="gpool", bufs=2))
            slp = ctx.enter_context(tc.tile_pool(name="slp", bufs=2))
            cell = ctx.enter_context(tc.tile_pool(name="cell", bufs=2))
            state = ctx.enter_context(tc.tile_pool(name="state", bufs=1))
            psum1 = ctx.enter_context(
                tc.tile_pool(name="psum1", bufs=4, space="PSUM"))
            psum2 = ctx.enter_context(
                tc.tile_pool(name="psum2", bufs=2, space="PSUM"))
            psumP = ctx.enter_context(
                tc.tile_pool(name="psumP", bufs=2, space="PSUM"))
            outp = ctx.enter_context(tc.tile_pool(name="outp", bufs=4))

            # small constants
            b1 = smalls.tile([HS, 4], F32)
            b2 = smalls.tile([HS, 4], F32)
            gb1 = smalls.tile([HS, 2], F32)
            gb2 = smalls.tile([HS, 2], F32)
            m1 = smalls.tile([HS, B], F32)
            m2 = smalls.tile([HS, B], F32)
            # small constants via Pool/SWDGE: keeps the HWDGE path clear for
            # the first x/weight loads
            for dst, src in ((b1, d_bias1), (b2, d_bias2), (gb1, d_gb1),
                             (gb2, d_gb2), (m1, d_m1T), (m2, d_m2T)):
                nc.gpsimd.dma_start(dst[:], src[:])

            def load_x(j):
                x_t = xpool.tile([P, NKE, B], F32R, tag="x", name=f"x_{j}")
                dma(x_t[:], d_xT[j][:].rearrange("k p b -> p k b"))
                return x_t

            # resident weights (wih1 + x0 first; wout last -- used at j=2)
            w_ih1 = wts.tile([P, 4, NKE, HS], F32R)
            w_hh1 = wts.tile([P, 4, NKH, HS], BF16)
            w_ih2 = wts.tile([P, 4, NKH, HS], BF16)
            w_hh2 = wts.tile([P, 4, NKH, HS], BF16)
            w_out = wts.tile([P, NKH, VS], F32R)
            for g in range(4):
                dma(w_ih1[:, g], d_wih1[g][:].rearrange("k p m -> p k m"))
            x0 = load_x(0)
            for dst, dram in ((w_hh1, d_whh1), (w_ih2, d_wih2),
                              (w_hh2, d_whh2)):
                for g in range(4):
                    dma(dst[:, g], dram[g][:].rearrange("k p m -> p k m"))
            # w_out (5MB, first used at j=2) is loaded inside iteration 1 so
            # the startup AG chain isn't stuck behind its transfers

            # persistent cell state
            c1 = state.tile([P, B], F32)
            c2 = state.tile([P, B], F32)
            nc.vector.memset(c1[:], 0.0)
            nc.vector.memset(c2[:], 0.0)

            def emit_ih1(j, x_t, stop):
                """x-side of layer-1 gates for step j.  Bank A=(i,g), B=(f,o).
                start on the bank's first matmul (gates 0/1), stop on its
                last (gates 2/3) unless deferred to the hh matmuls."""
                pgA = psum1.tile([P, 2, B], F32, tag="pg1", name=f"pgA1_{j}")
                pgB = psum1.tile([P, 2, B], F32, tag="pg1", name=f"pgB1_{j}")
                gloc = {0: (pgA, 0), 2: (pgA, 1), 1: (pgB, 0), 3: (pgB, 1)}
                for gate in (0, 2, 1, 3):
                    tl, sub = gloc[gate]
                    for k in range(NKE):
                        nc.tensor.matmul(
                            tl[:, sub], w_ih1[:, gate, k], x_t[:, k],
                            start=(k == 0 and gate in (0, 1)),
                            stop=(stop and k == NKE - 1 and gate in (2, 3)))
                return pgA, pgB

            def lstm_cell(pgA, pgB, bias, gbv, mask, c_st, h_out, y_out, t):
                """Gate activations + cell update + BatchNorm + dropout mask.
                Writes h into h_out and y into y_out (possibly != dtypes)."""
                i_t = cell.tile([P, B], F32, tag="i", name=f"i_{t}")
                f_t = cell.tile([P, B], F32, tag="f", name=f"f_{t}")
                g_t = cell.tile([P, B], F32, tag="g", name=f"g_{t}")
                o_t = cell.tile([P, B], F32, tag="o", name=f"o_{t}")
                nc.scalar.activation(i_t[:], pgA[:, 0], Sig, bias=bias[:, 0:1])
                nc.scalar.activation(g_t[:], pgA[:, 1], Tanh, bias=bias[:, 2:3])
                nc.scalar.activation(f_t[:], pgB[:, 0], Sig, bias=bias[:, 1:2])
                nc.scalar.activation(o_t[:], pgB[:, 1], Sig, bias=bias[:, 3:4])

                ig = cell.tile([P, B], F32, tag="ig", name=f"ig_{t}")
                nc.vector.tensor_mul(ig[:], i_t[:], g_t[:])
                fc = cell.tile([P, B], F32, tag="fc", name=f"fc_{t}")
                nc.vector.tensor_mul(fc[:], f_t[:], c_st[:])
                nc.vector.tensor_add(c_st[:], ig[:], fc[:])
                tnc = cell.tile([P, B], F32, tag="tc", name=f"tc_{t}")
                nc.scalar.activation(tnc[:], c_st[:], Tanh)
                nc.vector.tensor_mul(h_out, o_t[:], tnc[:])
                # BN stats over batch (free dim)
                st6 = cell.tile([P, 6], F32, tag="st", name=f"st_{t}")
                nc.vector.bn_stats(st6[:], h_out)
                mv = cell.tile([P, 2], F32, tag="mv", name=f"mv_{t}")
                nc.vector.bn_aggr(mv[:], st6[:])
                # rstd = rsqrt(var + eps), DVE-only fast-inverse-sqrt + NR
                v_t = cell.tile([P, 1], F32, tag="vv", name=f"vv_{t}")
                nc.vector.tensor_scalar_add(v_t[:], mv[:, 1:2], BN_EPS)
                r_a = cell.tile([P, 1], F32, tag="ra", name=f"ra_{t}")
                r_b = cell.tile([P, 1], F32, tag="rb", name=f"rb_{t}")
                ui = cell.tile([P, 1], I32, tag="ui", name=f"ui_{t}")
                nc.vector.tensor_scalar(ui[:], v_t[:].bitcast(I32), 1, None,
                                        op0=mybir.AluOpType.logical_shift_right)
                nc.vector.tensor_scalar(r_a[:].bitcast(I32), ui[:],
                                        -1, 0x5F3759DF,
                                        op0=mybir.AluOpType.mult,
                                        op1=mybir.AluOpType.add)
                rr = cell.tile([P, 1], F32, tag="rr", name=f"rr_{t}")
                ww = cell.tile([P, 1], F32, tag="ww", name=f"ww_{t}")
                r_cur, r_nxt = r_a, r_b
                for it in range(NR_ITERS):
                    nc.vector.tensor_mul(rr[:], r_cur[:], r_cur[:])
                    nc.vector.scalar_tensor_tensor(
                        ww[:], rr[:], -0.5, v_t[:],
                        op0=mybir.AluOpType.mult, op1=mybir.AluOpType.mult)
                    nc.vector.scalar_tensor_tensor(
                        r_nxt[:], ww[:], 1.5, r_cur[:],
                        op0=mybir.AluOpType.add, op1=mybir.AluOpType.mult)
                    r_cur, r_nxt = r_nxt, r_cur
                a_v = cell.tile([P, 1], F32, tag="av", name=f"av_{t}")
                nc.vector.tensor_mul(a_v[:], r_cur[:], gbv[:, 0:1])
                ma = cell.tile([P, 1], F32, tag="ma", name=f"ma_{t}")
                nc.vector.tensor_mul(ma[:], mv[:, 0:1], a_v[:])
                b_v = cell.tile([P, 1], F32, tag="bv", name=f"bv_{t}")
                nc.vector.tensor_sub(b_v[:], gbv[:, 1:2], ma[:])
                yt = cell.tile([P, B], F32, tag="yt", name=f"yt_{t}")
                nc.scalar.activation(yt[:], h_out, Ident,
                                     bias=b_v[:], scale=a_v[:])
                nc.vector.tensor_mul(y_out, yt[:], mask[:])

            def project(tp, y2g):
                """Projection of step tp (y2g: [P, NCORE, B] f32r)."""
                for bh in range(2):
                    for n, (noff, nlen) in enumerate(NCHUNKS):
                        pp = psumP.tile([P, 512], F32, tag="pp",
                                        name=f"pp_{tp}_{bh}_{n}")
                        for k in range(NKH):
                            nc.tensor.matmul(
                                pp[:, 0:nlen],
                                y2g[:, k, bh * P:(bh + 1) * P],
                                w_out[:, k, noff:noff + nlen],
                                start=(k == 0), stop=(k == NKH - 1))
                        o_sb = outp.tile([P, 512], F32, tag="osb",
                                         name=f"osb_{tp}_{bh}_{n}")
                        nc.scalar.activation(
                            o_sb[:, 0:nlen], pp[:, 0:nlen],
                            mybir.ActivationFunctionType.Copy)
                        nc.gpsimd.dma_start(
                            d_out_r[bh * P:(bh + 1) * P, tp,
                                    noff:noff + nlen], o_sb[:, 0:nlen])

            # ---------------- main loop ----------------
            pend1 = emit_ih1(0, x0, stop=True)
            gh_prev = None           # bf16 gathered (h1, y1, h2) of AG_{j-1}
            y2g_prev = None          # f32r gathered y2 of AG_{j-1}

            for j in range(T + 1):
                # ---- phase A: layer 2 of step j-1 ----
                # gate matmuls run in half-k waves so the first wave starts
                # as soon as the first half of the slot's gather DMA lands.
                if j >= 1:
                    pgA2 = psum2.tile([P, 2, B], F32, tag="pg2",
                                      name=f"pgA2_{j}")
                    pgB2 = psum2.tile([P, 2, B], F32, tag="pg2",
                                      name=f"pgB2_{j}")
                    gloc = {0: (pgA2, 0), 2: (pgA2, 1),
                            1: (pgB2, 0), 3: (pgB2, 1)}
                    if j >= 2:
                        for ks in (range(0, 2), range(2, 4),
                                   range(4, 6), range(6, 8)):
                            for gate in (0, 2, 1, 3):
                                tl, sub = gloc[gate]
                                for k in ks:
                                    nc.tensor.matmul(
                                        tl[:, sub], w_hh2[:, gate, k],
                                        gh_prev[:, 2, k],
                                        start=(k == 0 and gate in (0, 1)),
                                        stop=False)
                    for ks in (range(0, NKH // 2), range(NKH // 2, NKH)):
                        for gate in (0, 2, 1, 3):
                            tl, sub = gloc[gate]
                            for k in ks:
                                nc.tensor.matmul(
                                    tl[:, sub], w_ih2[:, gate, k],
                                    gh_prev[:, 1, k],
                                    start=(j == 1 and k == 0
                                           and gate in (0, 1)),
                                    stop=(k == NKH - 1 and gate in (2, 3)))
                    h2sl = slp.tile([P, B], BF16, tag="h2s", name=f"h2s_{j}")
                    y2sl = slp.tile([P, B], F32R, tag="y2s", name=f"y2s_{j}")
                    lstm_cell(pgA2, pgB2, b2, gb2, m2, c2,
                              h2sl[:], y2sl[:], f"2_{j}")
                    gi = agi[j % RING]
                    dma(gi[2 * P:3 * P], h2sl[:])
                    dma(gi[4 * P:6 * P].rearrange("(p h) b -> p (h b)", h=2),
                        y2sl[:].bitcast(BF16))

                # ---- layer 1 of step j ----
                if j <= T - 1:
                    if j >= 1:
                        pgA1, pgB1 = pend1
                        gloc = {0: (pgA1, 0), 2: (pgA1, 1),
                                1: (pgB1, 0), 3: (pgB1, 1)}
                        for ks in (range(0, NKH // 2), range(NKH // 2, NKH)):
                            for gate in (0, 2, 1, 3):
                                tl, sub = gloc[gate]
                                for k in ks:
                                    nc.tensor.matmul(
                                        tl[:, sub], w_hh1[:, gate, k],
                                        gh_prev[:, 0, k], start=False,
                                        stop=(k == NKH - 1
                                              and gate in (2, 3)))
                    hy1 = slp.tile([P, 2, B], BF16, tag="hy1", name=f"hy1_{j}")
                    lstm_cell(pend1[0], pend1[1], b1, gb1, m1, c1,
                              hy1[:, 0], hy1[:, 1], f"1_{j}")
                    dma(agi[j % RING][0:2 * P].rearrange(
                        "(s p) b -> p s b", s=2), hy1[:])

                # ---- phase B: AllGather AG_j ----
                gi = agi[j % RING]
                go = ago[j % RING]
                if FAKE_AG:
                    # thin stand-in: creates the agi->ago ordering dependency
                    # without the (collective-engine, not DMA) traffic
                    dma(go[0:AGROWS, 0:4], gi[:, 0:4])
                else:
                    nc.gpsimd.collective_compute(
                        "AllGather", mybir.AluOpType.bypass,
                        ins=[gi[:].opt()], outs=[go[:].opt()],
                        replica_groups=[list(range(NCORE))])
                # bf16 slots view: s in {0:h1, 1:y1, 2:h2}
                agor = go[:].rearrange("(c s p) b -> s p c b", s=6, p=P)
                # y2 f32-byte-image view: [u, p, c, (h b)]; u=2 selects
                # rows [4P:6P] of each core's block
                agory = go[:].rearrange("(c u p h) b -> u p c (h b)",
                                        u=3, p=P, h=2)
                gh = gpool.tile([P, 3, NCORE, B], BF16, tag="gh",
                                name=f"gh_{j}")
                y2g = gpool.tile([P, NCORE, B], F32R, tag="y2g",
                                 name=f"y2g_{j}")
                # consumption priority: h2 (L2 gates) > y1 (ih2) > h1 (hh1)
                # > y2 (projection, one whole step of slack); half-c splits
                # so the first gate wave starts after half a slot lands.
                hk = NCORE // 2
                qk = NCORE // 4
                if j == 0:
                    slots = (1, 0)
                elif j == T:
                    slots = ()
                else:
                    slots = (2, 1, 0)
                for s in slots:
                    if s == 2:
                        # quarter-split: first gate wave unblocks earliest
                        dma(gh[:, s, 0:qk], agor[s][:, 0:qk])
                        dma(gh[:, s, qk:hk], agor[s][:, qk:hk])
                    else:
                        dma(gh[:, s, 0:hk], agor[s][:, 0:hk])
                    dma(gh[:, s, hk:], agor[s][:, hk:])
                if j >= 1:
                    dma(y2g[:].bitcast(BF16), agory[2])

                # ---- phase C: AG-independent PE work fills the wait ----
                if j >= 2:
                    project(j - 2, y2g_prev)
                if j <= T - 2:
                    x_nxt = load_x(j + 1)
                    pend1 = emit_ih1(j + 1, x_nxt, stop=False)

                gh_prev = gh
                y2g_prev = y2g

            # tail projection for the last step
            project(T - 1, y2g_prev)

    nc.finalize()
    return nc


def _prep_inputs(features, captions, lengths, embed_table,
                 W_ih1, W_hh1, b_ih1, b_hh1, gamma1, beta1, mask1,
                 W_ih2, W_hh2, b_ih2, b_hh2, gamma2, beta2, mask2,
                 W_out, b_out):
    f32 = np.float32
    features = np.asarray(features, f32)
    captions = np.asarray(captions)
    embed_table = np.asarray(embed_table, f32)

    # x sequence [T, B, E] -> xT [T, NKE, P, B]
    x = np.empty((L + 1, B, E), f32)
    x[0] = features
    x[1:] = embed_table[captions].transpose(1, 0, 2)
    x = x[:T]
    xT = _fp32r_round(
        np.ascontiguousarray(x.transpose(0, 2, 1).reshape(T, NKE, P, B)))

    def wslice(Wf, c, K, rnd):
        # Wf [4H, K] -> per-core [4, K//P, P, HS] lhsT blocks
        Wg = np.asarray(Wf, f32).reshape(4, H, K)[:, c * HS:(c + 1) * HS, :]
        return rnd(np.ascontiguousarray(
            Wg.transpose(0, 2, 1).reshape(4, K // P, P, HS)))

    bsum1 = (np.asarray(b_ih1, f32) + np.asarray(b_hh1, f32)).reshape(4, H)
    bsum2 = (np.asarray(b_ih2, f32) + np.asarray(b_hh2, f32)).reshape(4, H)
    WoT = np.ascontiguousarray(np.asarray(W_out, f32).T)  # [H, V]

    in_maps = []
    for c in range(NCORE):
        u = slice(c * HS, (c + 1) * HS)
        v = slice(c * VS, (c + 1) * VS)
        in_maps.append({
            "xT": xT,
            "wih1": wslice(W_ih1, c, E, _fp32r_round),
            "whh1": wslice(W_hh1, c, H, _to_bf16),
            "wih2": wslice(W_ih2, c, H, _to_bf16),
            "whh2": wslice(W_hh2, c, H, _to_bf16),
            "woutT": _fp32r_round(np.ascontiguousarray(
                WoT[:, v].reshape(NKH, P, VS))),
            "bias1": np.ascontiguousarray(bsum1[:, u].T),
            "bias2": np.ascontiguousarray(bsum2[:, u].T),
            "gb1": np.ascontiguousarray(
                np.stack([np.asarray(gamma1, f32)[u],
                          np.asarray(beta1, f32)[u]], axis=1)),
            "gb2": np.ascontiguousarray(
                np.stack([np.asarray(gamma2, f32)[u],
                          np.asarray(beta2, f32)[u]], axis=1)),
            "m1T": np.ascontiguousarray(np.asarray(mask1, f32).T[u]),
            "m2T": np.ascontiguousarray(np.asarray(mask2, f32).T[u]),
        })
    return in_maps, np.asarray(b_out, f32)


def kernel(**inputs):
    global LAST_EXEC_NS
    if "nc" not in _CACHE:
        _CACHE["nc"] = build_bass()
    nc = _CACHE["nc"]

    in_maps, b_out = _prep_inputs(**inputs)
    trace = os.environ.get("TRN_KERNEL_TRACE", "0") == "1"
    res = run_bass_kernel_spmd(nc, in_maps, core_ids=list(range(NCORE)),
                               trace=trace)
    LAST_EXEC_NS = res.exec_time_ns
    out = np.concatenate([res.results[c]["out"] for c in range(NCORE)], axis=1)
    if b_out.any():
        out = out + b_out[None, :]
    return out


# revision 21
# speedup vs baseline: 1.7330x; 1.0210x over previous
"""Trainium2 Bass kernel for nn_Decoder (2-layer LSTM decoder with BatchNorm +
LockedDropout + vocab projection), tensor-parallel over the hidden dim across
8 NeuronCores.

Contract: kernel(**inputs) takes FULL inputs (as produced by setup_inputs())
and returns the FULL [B*T, V] float32 output.

Sharding / schedule (v3):
  - Each core owns a 128-unit slice of the hidden dim for BOTH LSTM layers
    (full batch B=256 moving dim, exact BN stats) and a 1250-wide slice of
    the vocab projection.
  - ONE AllGather per step (22 total): AG_j carries (h1_j, y1_j, h2_{j-1},
    y2_{j-1}).  Layer 2 of step j runs after AG_j lands, together with
    layer 1 of step j+1 -> a single collective round trip per step.
  - The vocab projection of step j-1 is emitted right after AG_j launches,
    so the PE has ~10us of AG-independent work during every collective
    wait (keeps the PE pstate ramped, hides AG latency).
  - Transport dtypes: h1/y1/h2 ride the collective in bf16 (halves the
    gather traffic; they feed gate matmuls as the *moving* operand).  y2
    rides in f32r, byte-packed into the bf16 buffer, because it is the
    *stationary* operand of the projection (f32r stationary emits no
    Ldweights instruction and keeps full precision on the output path).
  - Gather-in DMAs are per-slot, priority-ordered (h2, y1, h1, y2), with
    half-k splits so layer-2 gates start after half a slot lands.
  - PSUM->SBUF projection copies run on the ACT engine; projection output
    DMAs are issued from the Pool queue (SWDGE) to keep the shared HWDGE
    path free for x-loads and gathers.
"""

import contextlib
import os
import sys

sys.path.insert(0, "/opt/trn_rl_repo")

import ml_dtypes
import numpy as np

import concourse.bass as bass
import concourse.tile as tile
from concourse import bacc, mybir
from concourse.bass_utils import run_bass_kernel_spmd

F32 = mybir.dt.float32
F32R = mybir.dt.float32r
BF16 = mybir.dt.bfloat16

B, L, E, H, V = 256, 20, 512, 1024, 10000
T = int(os.environ.get("TRN_T", L + 1))
NCORE = 8
P = 128
HS = H // NCORE          # 128 hidden units per core per layer
VS = V // NCORE          # 1250 vocab slots per core
NKE = E // P             # 4 k-tiles over E
NKH = H // P             # 8 k-tiles over H
BN_EPS = 1e-5
RING = 3
NR_ITERS = int(os.environ.get("TRN_NR", "1"))
# projection N-chunks of VS=1250 (even lengths; fp32r full rate needs >=256)
NCHUNKS = [(0, 418), (418, 416), (834, 416)]
# agi rows (bf16 [6P, B]): h1 [0:P], y1 [P:2P], h2 [2P:3P], pad [3P:4P],
# y2 f32-byte-image [4P:6P]
AGROWS = 6 * P

LAST_EXEC_NS = None
# TRN_FAKE_AG=1 replaces collectives with a local DMA (timing-model runs only)
FAKE_AG = os.environ.get("TRN_FAKE_AG", "0") == "1"

_CACHE = {}


def _fp32r_round(x):
    """Round fp32 -> nearest fp32r value (sum of two bf16s)."""
    hi = x.astype(ml_dtypes.bfloat16).astype(np.float32)
    lo = (x - hi).astype(ml_dtypes.bfloat16).astype(np.float32)
    return hi + lo


def _to_bf16(x):
    return x.astype(ml_dtypes.bfloat16)


def build_bass():
    nc = bacc.Bacc("TRN2", target_bir_lowering=False, num_devices=NCORE)

    # ---------------- DRAM I/O ----------------
    d_xT = nc.dram_tensor("xT", [T, NKE, P, B], F32R, kind="ExternalInput")
    d_wih1 = nc.dram_tensor("wih1", [4, NKE, P, HS], F32R, kind="ExternalInput")
    d_whh1 = nc.dram_tensor("whh1", [4, NKH, P, HS], BF16, kind="ExternalInput")
    d_wih2 = nc.dram_tensor("wih2", [4, NKH, P, HS], BF16, kind="ExternalInput")
    d_whh2 = nc.dram_tensor("whh2", [4, NKH, P, HS], BF16, kind="ExternalInput")
    d_woutT = nc.dram_tensor("woutT", [NKH, P, VS], F32R, kind="ExternalInput")
    d_bias1 = nc.dram_tensor("bias1", [HS, 4], F32, kind="ExternalInput")
    d_bias2 = nc.dram_tensor("bias2", [HS, 4], F32, kind="ExternalInput")
    d_gb1 = nc.dram_tensor("gb1", [HS, 2], F32, kind="ExternalInput")
    d_gb2 = nc.dram_tensor("gb2", [HS, 2], F32, kind="ExternalInput")
    d_m1T = nc.dram_tensor("m1T", [HS, B], F32, kind="ExternalInput")
    d_m2T = nc.dram_tensor("m2T", [HS, B], F32, kind="ExternalInput")
    d_out = nc.dram_tensor("out", [B * T, VS], F32, kind="ExternalOutput")
    # out rows are (b, t) packed; view for per-(t, b-block) strided writes
    d_out_r = d_out[:].rearrange("(b t) v -> b t v", t=T)

    agi = [nc.dram_tensor(f"agi{r}", [AGROWS, B], BF16, kind="Internal")
           for r in range(RING)]
    ago = [nc.dram_tensor(f"ago{r}", [AGROWS * NCORE, B], BF16,
                          kind="Internal", addr_space="Shared")
           for r in range(RING)]

    dma = nc.sync.dma_start
    Sig = mybir.ActivationFunctionType.Sigmoid
    Tanh = mybir.ActivationFunctionType.Tanh
    Ident = mybir.ActivationFunctionType.Identity
    I32 = mybir.dt.int32

    with tile.TileContext(nc) as tc:
        with contextlib.ExitStack() as ctx:
            smalls = ctx.enter_context(tc.tile_pool(name="smalls", bufs=1))
            wts = ctx.enter_context(tc.tile_pool(name="wts", bufs=1))
            xpool = ctx.enter_context(tc.tile_pool(name="xpool", bufs=2))
            gpool = ctx.enter_context(tc.tile_pool(name="gpool", bufs=2))
            slp = ctx.enter_context(tc.tile_pool(name="slp", bufs=2))
            cell = ctx.enter_context(tc.tile_pool(name="cell", bufs=2))
            state = ctx.enter_context(tc.tile_pool(name="state", bufs=1))
            psum1 = ctx.enter_context(
                tc.tile_pool(name="psum1", bufs=4, space="PSUM"))
            psum2 = ctx.enter_context(
                tc.tile_pool(name="psum2", bufs=2, space="PSUM"))
            psumP = ctx.enter_context(
                tc.tile_pool(name="psumP", bufs=2, space="PSUM"))
            outp = ctx.enter_context(tc.tile_pool(name="outp", bufs=4))

            # small constants
            b1 = smalls.tile([HS, 4], F32)
            b2 = smalls.tile([HS, 4], F32)
            gb1 = smalls.tile([HS, 2], F32)
            gb2 = smalls.tile([HS, 2], F32)
            m1 = smalls.tile([HS, B], F32)
            m2 = smalls.tile([HS, B], F32)
            # small constants via Pool/SWDGE: keeps the HWDGE path clear for
            # the first x/weight loads
            for dst, src in ((b1, d_bias1), (b2, d_bias2), (gb1, d_gb1),
                             (gb2, d_gb2), (m1, d_m1T), (m2, d_m2T)):
                nc.gpsimd.dma_start(dst[:], src[:])

            def load_x(j):
                x_t = xpool.tile([P, NKE, B], F32R, tag="x", name=f"x_{j}")
                dma(x_t[:], d_xT[j][:].rearrange("k p b -> p k b"))
                return x_t

            # resident weights (wih1 + x0 first; wout last -- used at j=2)
            w_ih1 = wts.tile([P, 4, NKE, HS], F32R)
            w_hh1 = wts.tile([P, 4, NKH, HS], BF16)
            w_ih2 = wts.tile([P, 4, NKH, HS], BF16)
            w_hh2 = wts.tile([P, 4, NKH, HS], BF16)
            w_out = wts.tile([P, NKH, VS], F32R)
            # pre-loop loads: only what step 0 + hh1(1) need.  wih2/whh2
            # are emitted inside iteration 0 and w_out inside iteration 1
            # (SP queue is in-order, so later emission = later dispatch =
            # the startup AllGather chain is not stuck behind them).
            for g in range(4):
                dma(w_ih1[:, g], d_wih1[g][:].rearrange("k p m -> p k m"))
            x0 = load_x(0)
            for g in range(4):
                dma(w_hh1[:, g], d_whh1[g][:].rearrange("k p m -> p k m"))

            # persistent cell state
            c1 = state.tile([P, B], F32)
            c2 = state.tile([P, B], F32)
            nc.vector.memset(c1[:], 0.0)
            nc.vector.memset(c2[:], 0.0)

            def emit_ih1(j, x_t, stop):
                """x-side of layer-1 gates for step j.  Bank A=(i,g), B=(f,o).
                start on the bank's first matmul (gates 0/1), stop on its
                last (gates 2/3) unless deferred to the hh matmuls."""
                pgA = psum1.tile([P, 2, B], F32, tag="pg1", name=f"pgA1_{j}")
                pgB = psum1.tile([P, 2, B], F32, tag="pg1", name=f"pgB1_{j}")
                gloc = {0: (pgA, 0), 2: (pgA, 1), 1: (pgB, 0), 3: (pgB, 1)}
                for gate in (0, 2, 1, 3):
                    tl, sub = gloc[gate]
                    for k in range(NKE):
                        nc.tensor.matmul(
                            tl[:, sub], w_ih1[:, gate, k], x_t[:, k],
                            start=(k == 0 and gate in (0, 1)),
                            stop=(stop and k == NKE - 1 and gate in (2, 3)))
                return pgA, pgB

            def lstm_cell(pgA, pgB, bias, gbv, mask, c_st, h_out, y_out, t):
                """Gate activations + cell update + BatchNorm + dropout mask.
                Writes h into h_out and y into y_out (possibly != dtypes)."""
                i_t = cell.tile([P, B], F32, tag="i", name=f"i_{t}")
                f_t = cell.tile([P, B], F32, tag="f", name=f"f_{t}")
                g_t = cell.tile([P, B], F32, tag="g", name=f"g_{t}")
                o_t = cell.tile([P, B], F32, tag="o", name=f"o_{t}")
                nc.scalar.activation(i_t[:], pgA[:, 0], Sig, bias=bias[:, 0:1])
                nc.scalar.activation(g_t[:], pgA[:, 1], Tanh, bias=bias[:, 2:3])
                nc.scalar.activation(f_t[:], pgB[:, 0], Sig, bias=bias[:, 1:2])
                nc.scalar.activation(o_t[:], pgB[:, 1], Sig, bias=bias[:, 3:4])

                ig = cell.tile([P, B], F32, tag="ig", name=f"ig_{t}")
                nc.vector.tensor_mul(ig[:], i_t[:], g_t[:])
                fc = cell.tile([P, B], F32, tag="fc", name=f"fc_{t}")
                nc.vector.tensor_mul(fc[:], f_t[:], c_st[:])
                nc.vector.tensor_add(c_st[:], ig[:], fc[:])
                tnc = cell.tile([P, B], F32, tag="tc", name=f"tc_{t}")
                nc.scalar.activation(tnc[:], c_st[:], Tanh)
                nc.vector.tensor_mul(h_out, o_t[:], tnc[:])
                # BN stats over batch (free dim)
                st6 = cell.tile([P, 6], F32, tag="st", name=f"st_{t}")
                nc.vector.bn_stats(st6[:], h_out)
                mv = cell.tile([P, 2], F32, tag="mv", name=f"mv_{t}")
                nc.vector.bn_aggr(mv[:], st6[:])
                # rstd = rsqrt(var + eps), DVE-only fast-inverse-sqrt + NR
                v_t = cell.tile([P, 1], F32, tag="vv", name=f"vv_{t}")
                nc.vector.tensor_scalar_add(v_t[:], mv[:, 1:2], BN_EPS)
                r_a = cell.tile([P, 1], F32, tag="ra", name=f"ra_{t}")
                r_b = cell.tile([P, 1], F32, tag="rb", name=f"rb_{t}")
                ui = cell.tile([P, 1], I32, tag="ui", name=f"ui_{t}")
                nc.vector.tensor_scalar(ui[:], v_t[:].bitcast(I32), 1, None,
                                        op0=mybir.AluOpType.logical_shift_right)
                nc.vector.tensor_scalar(r_a[:].bitcast(I32), ui[:],
                                        -1, 0x5F3759DF,
                                        op0=mybir.AluOpType.mult,
                                        op1=mybir.AluOpType.add)
                rr = cell.tile([P, 1], F32, tag="rr", name=f"rr_{t}")
                ww = cell.tile([P, 1], F32, tag="ww", name=f"ww_{t}")
                r_cur, r_nxt = r_a, r_b
                for it in range(NR_ITERS):
                    nc.vector.tensor_mul(rr[:], r_cur[:], r_cur[:])
                    nc.vector.scalar_tensor_tensor(
                        ww[:], rr[:], -0.5, v_t[:],
                        op0=mybir.AluOpType.mult, op1=mybir.AluOpType.mult)
                    nc.vector.scalar_tensor_tensor(
                        r_nxt[:], ww[:], 1.5, r_cur[:],
                        op0=mybir.AluOpType.add, op1=mybir.AluOpType.mult)
                    r_cur, r_nxt = r_nxt, r_cur
                a_v = cell.tile([P, 1], F32, tag="av", name=f"av_{t}")
                nc.vector.tensor_mul(a_v[:], r_cur[:], gbv[:, 0:1])
                ma = cell.tile([P, 1], F32, tag="ma", name=f"ma_{t}")
                nc.vector.tensor_mul(ma[:], mv[:, 0:1], a_v[:])
                b_v = cell.tile([P, 1], F32, tag="bv", name=f"bv_{t}")
                nc.vector.tensor_sub(b_v[:], gbv[:, 1:2], ma[:])
                yt = cell.tile([P, B], F32, tag="yt", name=f"yt_{t}")
                nc.scalar.activation(yt[:], h_out, Ident,
                                     bias=b_v[:], scale=a_v[:])
                nc.vector.tensor_mul(y_out, yt[:], mask[:])

            def project(tp, y2g):
                """Projection of step tp (y2g: [P, NCORE, B] f32r)."""
                for bh in range(2):
                    for n, (noff, nlen) in enumerate(NCHUNKS):
                        pp = psumP.tile([P, 512], F32, tag="pp",
                                        name=f"pp_{tp}_{bh}_{n}")
                        for k in range(NKH):
                            nc.tensor.matmul(
                                pp[:, 0:nlen],
                                y2g[:, k, bh * P:(bh + 1) * P],
                                w_out[:, k, noff:noff + nlen],
                                start=(k == 0), stop=(k == NKH - 1))
                        o_sb = outp.tile([P, 512], F32, tag="osb",
                                         name=f"osb_{tp}_{bh}_{n}")
                        nc.scalar.activation(
                            o_sb[:, 0:nlen], pp[:, 0:nlen],
                            mybir.ActivationFunctionType.Copy)
                        nc.gpsimd.dma_start(
                            d_out_r[bh * P:(bh + 1) * P, tp,
                                    noff:noff + nlen], o_sb[:, 0:nlen])

            # ---------------- main loop ----------------
            pend1 = emit_ih1(0, x0, stop=True)
            gh_prev = None           # bf16 gathered (h1, y1, h2) of AG_{j-1}
            y2g_prev = None          # f32r gathered y2 of AG_{j-1}

            for j in range(T + 1):
                # ---- phase A: layer 2 of step j-1 ----
                # gate matmuls run in half-k waves so the first wave starts
                # as soon as the first half of the slot's gather DMA lands.
                if j >= 1:
                    pgA2 = psum2.tile([P, 2, B], F32, tag="pg2",
                                      name=f"pgA2_{j}")
                    pgB2 = psum2.tile([P, 2, B], F32, tag="pg2",
                                      name=f"pgB2_{j}")
                    gloc = {0: (pgA2, 0), 2: (pgA2, 1),
                            1: (pgB2, 0), 3: (pgB2, 1)}
                    if j >= 2:
                        for ks in (range(0, 2), range(2, 4),
                                   range(4, 6), range(6, 8)):
                            for gate in (0, 2, 1, 3):
                                tl, sub = gloc[gate]
                                for k in ks:
                                    nc.tensor.matmul(
                                        tl[:, sub], w_hh2[:, gate, k],
                                        gh_prev[:, 2, k],
                                        start=(k == 0 and gate in (0, 1)),
                                        stop=False)
                    for ks in (range(0, NKH // 2), range(NKH // 2, NKH)):
                        for gate in (0, 2, 1, 3):
                            tl, sub = gloc[gate]
                            for k in ks:
                                nc.tensor.matmul(
                                    tl[:, sub], w_ih2[:, gate, k],
                                    gh_prev[:, 1, k],
                                    start=(j == 1 and k == 0
                                           and gate in (0, 1)),
                                    stop=(k == NKH - 1 and gate in (2, 3)))
                    h2sl = slp.tile([P, B], BF16, tag="h2s", name=f"h2s_{j}")
                    y2sl = slp.tile([P, B], F32R, tag="y2s", name=f"y2s_{j}")
                    lstm_cell(pgA2, pgB2, b2, gb2, m2, c2,
                              h2sl[:], y2sl[:], f"2_{j}")
                    gi = agi[j % RING]
                    dma(gi[2 * P:3 * P], h2sl[:])
                    dma(gi[4 * P:6 * P].rearrange("(p h) b -> p (h b)", h=2),
                        y2sl[:].bitcast(BF16))

                # ---- layer 1 of step j ----
                if j <= T - 1:
                    if j >= 1:
                        pgA1, pgB1 = pend1
                        gloc = {0: (pgA1, 0), 2: (pgA1, 1),
                                1: (pgB1, 0), 3: (pgB1, 1)}
                        for ks in (range(0, NKH // 2), range(NKH // 2, NKH)):
                            for gate in (0, 2, 1, 3):
                                tl, sub = gloc[gate]
                                for k in ks:
                                    nc.tensor.matmul(
                                        tl[:, sub], w_hh1[:, gate, k],
                                        gh_prev[:, 0, k], start=False,
                                        stop=(k == NKH - 1
                                              and gate in (2, 3)))
                    hy1 = slp.tile([P, 2, B], BF16, tag="hy1", name=f"hy1_{j}")
                    lstm_cell(pend1[0], pend1[1], b1, gb1, m1, c1,
                              hy1[:, 0], hy1[:, 1], f"1_{j}")
                    dma(agi[j % RING][0:2 * P].rearrange(
                        "(s p) b -> p s b", s=2), hy1[:])

                # ---- phase B: AllGather AG_j ----
                gi = agi[j % RING]
                go = ago[j % RING]
                if FAKE_AG:
                    # thin stand-in: creates the agi->ago ordering dependency
                    # without the (collective-engine, not DMA) traffic
                    dma(go[0:AGROWS, 0:4], gi[:, 0:4])
                else:
                    nc.gpsimd.collective_compute(
                        "AllGather", mybir.AluOpType.bypass,
                        ins=[gi[:].opt()], outs=[go[:].opt()],
                        replica_groups=[list(range(NCORE))])
                # bf16 slots view: s in {0:h1, 1:y1, 2:h2}
                agor = go[:].rearrange("(c s p) b -> s p c b", s=6, p=P)
                # y2 f32-byte-image view: [u, p, c, (h b)]; u=2 selects
                # rows [4P:6P] of each core's block
                agory = go[:].rearrange("(c u p h) b -> u p c (h b)",
                                        u=3, p=P, h=2)
                gh = None
                y2g = None
                if j <= T - 1:
                    gh = gpool.tile([P, 3, NCORE, B], BF16, tag="gh",
                                    name=f"gh_{j}")
                if j >= 1:
                    y2g = gpool.tile([P, NCORE, B], F32R, tag="y2g",
                                     name=f"y2g_{j}")
                # consumption priority: h2 (L2 gates) > y1 (ih2) > h1 (hh1)
                # > y2 (projection, one whole step of slack); half-c splits
                # so the first gate wave starts after half a slot lands.
                hk = NCORE // 2
                qk = NCORE // 4
                if j == 0:
                    slots = (1, 0)
                elif j == T:
                    slots = ()
                else:
                    slots = (2, 1, 0)
                for s in slots:
                    if s == 2:
                        # quarter-split: first gate wave unblocks earliest
                        dma(gh[:, s, 0:qk], agor[s][:, 0:qk])
                        dma(gh[:, s, qk:hk], agor[s][:, qk:hk])
                    else:
                        dma(gh[:, s, 0:hk], agor[s][:, 0:hk])
                    dma(gh[:, s, hk:], agor[s][:, hk:])
                if j >= 1:
                    dma(y2g[:].bitcast(BF16), agory[2])

                # ---- phase C: AG-independent PE work fills the wait ----
                if j >= 2:
                    project(j - 2, y2g_prev)
                if j <= T - 2:
                    x_nxt = load_x(j + 1)
                    pend1 = emit_ih1(j + 1, x_nxt, stop=False)
                if j == 0:
                    for dst, dram in ((w_ih2, d_wih2), (w_hh2, d_whh2)):
                        for g in range(4):
                            dma(dst[:, g],
                                dram[g][:].rearrange("k p m -> p k m"))
                elif j == 1:
                    # w_out lands during iterations 1-2, ahead of project(0)
                    for k in range(NKH):
                        dma(w_out[:, k], d_woutT[k][:])

                gh_prev = gh
                y2g_prev = y2g

            # tail projection for the last step
            project(T - 1, y2g_prev)

    nc.finalize()
    return nc


def _prep_inputs(features, captions, lengths, embed_table,
                 W_ih1, W_hh1, b_ih1, b_hh1, gamma1, beta1, mask1,
                 W_ih2, W_hh2, b_ih2, b_hh2, gamma2, beta2, mask2,
                 W_out, b_out):
    f32 = np.float32
    features = np.asarray(features, f32)
    captions = np.asarray(captions)
    embed_table = np.asarray(embed_table, f32)

    # x sequence [T, B, E] -> xT [T, NKE, P, B]
    x = np.empty((L + 1, B, E), f32)
    x[0] = features
    x[1:] = embed_table[captions].transpose(1, 0, 2)
    x = x[:T]
    xT = _fp32r_round(
        np.ascontiguousarray(x.transpose(0, 2, 1).reshape(T, NKE, P, B)))

    def wslice(Wf, c, K, rnd):
        # Wf [4H, K] -> per-core [4, K//P, P, HS] lhsT blocks
        Wg = np.asarray(Wf, f32).reshape(4, H, K)[:, c * HS:(c + 1) * HS, :]
        return rnd(np.ascontiguousarray(
            Wg.transpose(0, 2, 1).reshape(4, K // P, P, HS)))

    bsum1 = (np.asarray(b_ih1, f32) + np.asarray(b_hh1, f32)).reshape(4, H)
    bsum2 = (np.asarray(b_ih2, f32) + np.asarray(b_hh2, f32)).reshape(4, H)
    WoT = np.ascontiguousarray(np.asarray(W_out, f32).T)  # [H, V]

    in_maps = []
    for c in range(NCORE):
        u = slice(c * HS, (c + 1) * HS)
        v = slice(c * VS, (c + 1) * VS)
        in_maps.append({
            "xT": xT,
            "wih1": wslice(W_ih1, c, E, _fp32r_round),
            "whh1": wslice(W_hh1, c, H, _to_bf16),
            "wih2": wslice(W_ih2, c, H, _to_bf16),
            "whh2": wslice(W_hh2, c, H, _to_bf16),
            "woutT": _fp32r_round(np.ascontiguousarray(
                WoT[:, v].reshape(NKH, P, VS))),
            "bias1": np.ascontiguousarray(bsum1[:, u].T),
            "bias2": np.ascontiguousarray(bsum2[:, u].T),
            "gb1": np.ascontiguousarray(
                np.stack([np.asarray(gamma1, f32)[u],
                          np.asarray(beta1, f32)[u]], axis=1)),
            "gb2": np.ascontiguousarray(
                np.stack([np.asarray(gamma2, f32)[u],
                          np.asarray(beta2, f32)[u]], axis=1)),
            "m1T": np.ascontiguousarray(np.asarray(mask1, f32).T[u]),
            "m2T": np.ascontiguousarray(np.asarray(mask2, f32).T[u]),
        })
    return in_maps, np.asarray(b_out, f32)


def kernel(**inputs):
    global LAST_EXEC_NS
    if "nc" not in _CACHE:
        _CACHE["nc"] = build_bass()
    nc = _CACHE["nc"]

    in_maps, b_out = _prep_inputs(**inputs)
    trace = os.environ.get("TRN_KERNEL_TRACE", "0") == "1"
    res = run_bass_kernel_spmd(nc, in_maps, core_ids=list(range(NCORE)),
                               trace=trace)
    LAST_EXEC_NS = res.exec_time_ns
    out = np.concatenate([res.results[c]["out"] for c in range(NCORE)], axis=1)
    if b_out.any():
        out = out + b_out[None, :]
    return out
